# revision 10
# baseline (speedup 1.0000x reference)
"""Trainium2 Bass kernel for nn_MetaPN (hypernetwork MLP).

Math (per sample b):
  w1 = (pe @ W1w.T + b1w).reshape(2, D);  bb1 = pe @ W1b.T + b1b
  x1 = prelu(coods @ w1 + bb1)
  x2 = prelu(sum_d x1[d] * w2[d, :] + bb2),  w2 = (pe @ W2w.T + b2w).reshape(D, D)
  x3 = sum_d x2[d] * w3[d, :] + bb3,         w3 = (pe @ W3w.T + b3w).reshape(D, DT)

Kernel strategy (pure data parallel over batch, 8 cores x 512 samples):
  - Weight-gen matmuls H_d[b,e] = sum_k pe[b,k] * V2[d,k,e] on TensorE
    (stationary = pe^T chunks, moving = host-permuted V2 streamed from HBM,
    fp32r at N=256 -> 1 cycle/row).
  - Per-sample scaling S_d[b,e] = x1[b,d] * H_d[b,e] on VectorE/ScalarE
    via tensor_scalar with per-partition scalar (batch on partitions).
  - Accumulation x2[b,e] = sum_d S_d[b,e] on TensorE via identity-stationary
    matmuls accumulating in PSUM (start/stop flags).
  - All hypernetwork biases folded into extra matmul contraction rows.
"""

import os

import numpy as np

import concourse.bass as bass
from concourse import bacc
import concourse.mybir as mybir
from concourse.tile import TileContext
from concourse.bass_utils import run_bass_kernel_spmd

D = 256
DT = 64
B = 4096
NCORES = 8
BP = B // NCORES          # samples per core = 512
NBT = BP // 128           # batch tiles per core = 4
KC = 2                    # contraction chunks of 128 over k (=D=256)
ALPHA = 0.25              # PReLU alpha (nn.PReLU default from setup_inputs)

F32 = mybir.dt.float32
F32R = mybir.dt.float32r

# packed-constant column offsets (fp32 elements within [128, CTOT])
O_PET = 0                 # peT           [128, 2*512]
O_W1W = 1024              # W1w.T         [128, 2*512]
O_W1B = 2048              # W1b.T         [128, 2*256]
O_W2B = 2560              # W2b.T         [128, 2*256]
O_W3B = 3072              # W3b.T padded  [128, 2*256]
O_ID = 3584               # identity      [128, 128]
O_CT = 3712               # [ones; c0; c1] rows 0-2, per bt chunk [128, 512]
O_W1X = 4224              # [b1b; b1w_a; b1w_b] rows 0-2  [128, 256]
O_B2B = 4480              # b2b row 0     [128, 256]
O_B3B = 4736              # b3b padded row 0 [128, 256]
O_B2W = 4992              # b2w.reshape(D,D) kc-split   [128, 2*256]
O_B3W = 5504              # b3w.reshape(D,DT) kc-split  [128, 2*64]
CTOT = 5632

LAST_RESULTS = None       # BassKernelResults of the most recent run (for test.py)


def build_module():
    nc = bacc.Bacc("TRN2", target_bir_lowering=False)

    # ---- DRAM I/O ----
    const_d = nc.dram_tensor("CONST", [128, CTOT], F32R, kind="ExternalInput")
    cood_d = nc.dram_tensor("cood", [128, NBT * 2], F32, kind="ExternalInput")
    v2_d = nc.dram_tensor("V2", [D, D, D], F32R, kind="ExternalInput")
    v3_d = nc.dram_tensor("V3", [D // 4, D, D], F32R, kind="ExternalInput")
    out_d = nc.dram_tensor("out", [128, NBT * DT], F32, kind="ExternalOutput")
    dbg1_d = nc.dram_tensor("x1dbg", [128, NBT * D], F32, kind="ExternalOutput")
    dbg2_d = nc.dram_tensor("x2dbg", [128, NBT * D], F32, kind="ExternalOutput")

    with TileContext(nc) as tc:
        with (
            tc.tile_pool(name="const", bufs=1) as cp,
            tc.tile_pool(name="v2s", bufs=3) as v2p,
            tc.tile_pool(name="v3s", bufs=3) as v3p,
            tc.tile_pool(name="spool", bufs=8) as sp,
            tc.tile_pool(name="tmp", bufs=6) as tp,
            tc.tile_pool(name="hps", bufs=2, space="PSUM") as hp,
            tc.tile_pool(name="accps", bufs=1, space="PSUM") as accp,
            tc.tile_pool(name="l1ps", bufs=1, space="PSUM") as l1p,
        ):
            # ---- load constants / inputs to SBUF (2 DMAs total) ----
            c_s = cp.tile([128, CTOT], F32R)
            nc.sync.dma_start(out=c_s[:, :], in_=const_d[:, :])
            cood_s = cp.tile([128, NBT, 2], F32)
            nc.sync.dma_start(out=cood_s[:, :, :], in_=cood_d[:, :].rearrange("p (bt c) -> p bt c", bt=NBT))

            x1_s = cp.tile([128, NBT, D], F32)
            x1T_s = cp.tile([128, KC, BP], F32R)
            x2p_s = cp.tile([128, NBT, D], F32)
            x2pT_s = cp.tile([128, KC, BP], F32R)
            out_s = cp.tile([128, NBT, DT], F32)

            def petk(kc, bt):
                o = O_PET + kc * BP + bt * 128
                return c_s[:, o:o + 128]

            def w1wT(kc):
                o = O_W1W + kc * 2 * D
                return c_s[:, o:o + 2 * D]

            def seg2(base, kc):
                o = base + kc * D
                return c_s[:, o:o + D]

            ident = c_s[:, O_ID:O_ID + 128]
            ident_f32 = c_s[:, O_ID:O_ID + 128].bitcast(F32)

            def coodT3(bt):
                o = O_CT + bt * 128
                return c_s[0:3, o:o + 128]

            def ones1(bt):
                o = O_CT + bt * 128
                return c_s[0:1, o:o + 128]

            w1x = c_s[0:3, O_W1X:O_W1X + D]
            b2b = c_s[0:1, O_B2B:O_B2B + D]
            b3b = c_s[0:1, O_B3B:O_B3B + D]

            # ================= Layer 1 =================
            for bt in range(NBT):
                h1 = l1p.tile([128, 2 * D], F32, tag="h1")
                nc.tensor.matmul(h1[:, :], petk(0, bt), w1wT(0), start=True, stop=False)
                nc.tensor.matmul(h1[:, :], petk(1, bt), w1wT(1), start=False, stop=True)
                bb = l1p.tile([128, D], F32, tag="bb")
                nc.tensor.matmul(bb[:, :], petk(0, bt), seg2(O_W1B, 0), start=True, stop=False)
                nc.tensor.matmul(bb[:, :], petk(1, bt), seg2(O_W1B, 1), start=False, stop=False)
                nc.tensor.matmul(bb[:, :], coodT3(bt), w1x, start=False, stop=True)
                # x1 = prelu(c0 * h1a + c1 * h1b + bb)
                t0 = tp.tile([128, D], F32, tag="t0")
                t1 = tp.tile([128, D], F32, tag="t1")
                t2 = tp.tile([128, D], F32, tag="t2")
                nc.vector.tensor_scalar_mul(t0[:, :], h1[:, 0:D], cood_s[:, bt, 0:1])
                nc.scalar.activation(t1[:, :], h1[:, D:2 * D], mybir.ActivationFunctionType.Copy,
                                     scale=cood_s[:, bt, 1:2])
                nc.vector.tensor_tensor(t2[:, :], t0[:, :], t1[:, :], mybir.AluOpType.add)
                nc.vector.tensor_tensor(t0[:, :], t2[:, :], bb[:, :], mybir.AluOpType.add)
                nc.scalar.activation(x1_s[:, bt, :], t0[:, :], mybir.ActivationFunctionType.Prelu,
                                     alpha=ALPHA)

            # transpose x1 -> x1T (for the b2w bias term x1 @ B2)
            for bt in range(NBT):
                for dc in range(KC):
                    tr = l1p.tile([128, 128], F32, tag="bb")
                    nc.tensor.transpose(tr[:, :], x1_s[:, bt, dc * 128:(dc + 1) * 128], ident_f32)
                    nc.scalar.activation(x1T_s[:, dc, bt * 128:(bt + 1) * 128], tr[:, :],
                                         mybir.ActivationFunctionType.Copy)

            # ================= Layer 2 =================
            x2a = accp.tile([128, NBT, D], F32, tag="acc")
            for bt in range(NBT):
                nc.tensor.matmul(x2a[:, bt, :], petk(0, bt), seg2(O_W2B, 0), start=(bt % 2 == 0), stop=False)
                nc.tensor.matmul(x2a[:, bt, :], petk(1, bt), seg2(O_W2B, 1), start=False, stop=False)
                nc.tensor.matmul(x2a[:, bt, :], ones1(bt), b2b, start=False, stop=False)
                nc.tensor.matmul(x2a[:, bt, :], x1T_s[:, 0, bt * 128:(bt + 1) * 128],
                                 seg2(O_B2W, 0), start=False, stop=False)
                nc.tensor.matmul(x2a[:, bt, :], x1T_s[:, 1, bt * 128:(bt + 1) * 128],
                                 seg2(O_B2W, 1), start=False, stop=False)

            DBLK = 4  # d's per DMA chunk
            for dblk in range(D // DBLK):
                v2t = v2p.tile([128, DBLK, KC, D], F32R, tag="v2")
                nc.sync.dma_start(
                    out=v2t[:, :, :, :],
                    in_=v2_d[dblk * DBLK:(dblk + 1) * DBLK, :, :].rearrange(
                        "d (kc p) e -> p d kc e", kc=KC),
                )
                for dsub in range(DBLK):
                    d = dblk * DBLK + dsub
                    ht = hp.tile([128, NBT, D], F32, tag="H")
                    for bt in range(NBT):
                        nc.tensor.matmul(ht[:, bt, :], petk(0, bt), v2t[:, dsub, 0, :],
                                         start=(bt % 2 == 0), stop=False)
                        nc.tensor.matmul(ht[:, bt, :], petk(1, bt), v2t[:, dsub, 1, :],
                                         start=False, stop=(bt % 2 == 1))
                    for bt in range(NBT):
                        st = sp.tile([128, D], F32R, tag="S")
                        if bt % 2 == 0:
                            nc.vector.tensor_scalar_mul(st[:, :], ht[:, bt, :], x1_s[:, bt, d:d + 1])
                        else:
                            nc.scalar.activation(st[:, :], ht[:, bt, :],
                                                 mybir.ActivationFunctionType.Copy,
                                                 scale=x1_s[:, bt, d:d + 1])
                        nc.tensor.matmul(x2a[:, bt, :], ident, st[:, :],
                                         start=False, stop=(d == D - 1))

            for bt in range(NBT):
                nc.scalar.activation(x2p_s[:, bt, :], x2a[:, bt, :],
                                     mybir.ActivationFunctionType.Prelu, alpha=ALPHA)

            # transpose x2p -> x2pT (for the b3w bias term x2p @ B3)
            for bt in range(NBT):
                for dc in range(KC):
                    tr = l1p.tile([128, 128], F32, tag="bb")
                    nc.tensor.transpose(tr[:, :], x2p_s[:, bt, dc * 128:(dc + 1) * 128], ident_f32)
                    nc.scalar.activation(x2pT_s[:, dc, bt * 128:(bt + 1) * 128], tr[:, :],
                                         mybir.ActivationFunctionType.Copy)

            # ================= Layer 3 =================
            x3a = accp.tile([128, NBT, D], F32, tag="acc")
            b3w_cols = c_s[:, O_B3W:O_B3W + 2 * DT]
            for bt in range(NBT):
                nc.tensor.matmul(x3a[:, bt, :], petk(0, bt), seg2(O_W3B, 0), start=(bt % 2 == 0), stop=False)
                nc.tensor.matmul(x3a[:, bt, :], petk(1, bt), seg2(O_W3B, 1), start=False, stop=False)
                nc.tensor.matmul(x3a[:, bt, :], ones1(bt), b3b, start=False, stop=False)
                nc.tensor.matmul(x3a[:, bt, 0:DT], x2pT_s[:, 0, bt * 128:(bt + 1) * 128],
                                 b3w_cols[:, 0:DT], start=False, stop=False)
                nc.tensor.matmul(x3a[:, bt, 0:DT], x2pT_s[:, 1, bt * 128:(bt + 1) * 128],
                                 b3w_cols[:, DT:2 * DT], start=False, stop=False)

            NBLK = D // 4  # 4 d's per block, packed host-side in V3
            for blk in range(NBLK):
                v3t = v3p.tile([128, KC, D], F32R, tag="v3")
                nc.sync.dma_start(
                    out=v3t[:, :, :],
                    in_=v3_d[blk, :, :].rearrange("(kc p) e -> p kc e", kc=KC),
                )
                h3 = hp.tile([128, NBT, D], F32, tag="H")
                for bt in range(NBT):
                    nc.tensor.matmul(h3[:, bt, :], petk(0, bt), v3t[:, 0, :],
                                     start=(bt % 2 == 0), stop=False)
                    nc.tensor.matmul(h3[:, bt, :], petk(1, bt), v3t[:, 1, :],
                                     start=False, stop=(bt % 2 == 1))
                for bt in range(NBT):
                    s3 = sp.tile([128, D], F32R, tag="S")
                    for g in range(4):
                        dd = 4 * blk + g
                        if bt % 2 == 0:
                            nc.vector.tensor_scalar_mul(s3[:, g * DT:(g + 1) * DT],
                                                        h3[:, bt, g * DT:(g + 1) * DT],
                                                        x2p_s[:, bt, dd:dd + 1])
                        else:
                            nc.scalar.activation(s3[:, g * DT:(g + 1) * DT],
                                                 h3[:, bt, g * DT:(g + 1) * DT],
                                                 mybir.ActivationFunctionType.Copy,
                                                 scale=x2p_s[:, bt, dd:dd + 1])
                    nc.tensor.matmul(x3a[:, bt, :], ident, s3[:, :],
                                     start=False, stop=(blk == NBLK - 1))

            # combine the 4 column groups: x3 = g0 + g1 + g2 + g3
            for bt in range(NBT):
                u0 = tp.tile([128, DT], F32, tag="u0")
                u1 = tp.tile([128, DT], F32, tag="u1")
                u2 = tp.tile([128, DT], F32, tag="u2")
                nc.scalar.activation(u0[:, :], x3a[:, bt, 0:DT],
                                     mybir.ActivationFunctionType.Copy)
                nc.vector.tensor_tensor(u1[:, :], u0[:, :], x3a[:, bt, DT:2 * DT], mybir.AluOpType.add)
                nc.vector.tensor_tensor(u2[:, :], u1[:, :], x3a[:, bt, 2 * DT:3 * DT], mybir.AluOpType.add)
                nc.vector.tensor_tensor(out_s[:, bt, :], u2[:, :], x3a[:, bt, 3 * DT:4 * DT],
                                        mybir.AluOpType.add)

            nc.sync.dma_start(out=out_d[:, :], in_=out_s[:, :, :].rearrange("p bt t -> p (bt t)"))
            nc.sync.dma_start(out=dbg1_d[:, :], in_=x1_s[:, :, :].rearrange("p bt k -> p (bt k)"))
            nc.sync.dma_start(out=dbg2_d[:, :], in_=x2p_s[:, :, :].rearrange("p bt k -> p (bt k)"))

    nc.compile()
    return nc


def _kc_split(mat):
    """[256, F] -> [128, 2*F] with row p holding [chunk0(p), chunk1(p)]."""
    f = mat.shape[1]
    return np.ascontiguousarray(
        mat.reshape(KC, 128, f).transpose(1, 0, 2).reshape(128, KC * f))


def _prep_host(coods, pe, W1w, b1w, W1b, b1b, W2w, b2w, W2b, b2b, W3w, b3w, W3b, b3b):
    f = np.float32
    V2 = np.ascontiguousarray(W2w.reshape(D, D, D).transpose(0, 2, 1), dtype=f)
    V3 = np.ascontiguousarray(
        W3w.reshape(D // 4, 4, DT, D).transpose(0, 3, 1, 2).reshape(D // 4, D, 4 * DT), dtype=f)

    base = np.zeros((128, CTOT), dtype=f)
    base[:, O_W1W:O_W1W + 1024] = _kc_split(np.asarray(W1w.T, dtype=f))
    base[:, O_W1B:O_W1B + 512] = _kc_split(np.asarray(W1b.T, dtype=f))
    base[:, O_W2B:O_W2B + 512] = _kc_split(np.asarray(W2b.T, dtype=f))
    W3bTp = np.zeros((D, D), dtype=f)
    W3bTp[:, :DT] = np.asarray(W3b.T, dtype=f)
    base[:, O_W3B:O_W3B + 512] = _kc_split(W3bTp)
    base[:, O_ID:O_ID + 128] = np.eye(128, dtype=f)
    base[0, O_W1X:O_W1X + D] = b1b
    base[1, O_W1X:O_W1X + D] = b1w[:D]
    base[2, O_W1X:O_W1X + D] = b1w[D:]
    base[0, O_B2B:O_B2B + D] = b2b
    base[0, O_B3B:O_B3B + DT] = b3b
    base[:, O_B2W:O_B2W + 512] = _kc_split(np.asarray(b2w.reshape(D, D), dtype=f))
    base[:, O_B3W:O_B3W + 2 * DT] = _kc_split(np.asarray(b3w.reshape(D, DT), dtype=f))

    in_maps = []
    for i in range(NCORES):
        sl = slice(i * BP, (i + 1) * BP)
        pe_sh = np.asarray(pe[sl], dtype=f)         # [BP, D]
        cood_sh = np.asarray(coods[sl], dtype=f)    # [BP, 2]
        const = base.copy()
        const[:, O_PET:O_PET + KC * BP] = np.ascontiguousarray(
            pe_sh.T.reshape(KC, 128, BP).transpose(1, 0, 2).reshape(128, KC * BP))
        # [ones; c0; c1] rows, chunked per batch tile
        ct = np.zeros((128, NBT, 128), dtype=f)
        csp = cood_sh.reshape(NBT, 128, 2)
        ct[0, :, :] = 1.0
        ct[1] = csp[:, :, 0]
        ct[2] = csp[:, :, 1]
        const[:, O_CT:O_CT + NBT * 128] = ct.reshape(128, NBT * 128)
        cood_n = np.ascontiguousarray(
            cood_sh.reshape(NBT, 128, 2).transpose(1, 0, 2).reshape(128, NBT * 2))
        in_maps.append({"CONST": const, "cood": cood_n, "V2": V2, "V3": V3})
    return in_maps


def kernel(coods, pe, W1w, b1w, W1b, b1b, W2w, b2w, W2b, b2b,
           W3w, b3w, W3b, b3b, alpha):
    global LAST_RESULTS
    in_maps = _prep_host(coods, pe, W1w, b1w, W1b, b1b, W2w, b2w,
                         W2b, b2b, W3w, b3w, W3b, b3b)
    nc = build_module()
    trace = bool(int(os.environ.get("KERNEL_TRACE", "0")))
    res = run_bass_kernel_spmd(nc, in_maps, core_ids=list(range(NCORES)), trace=trace)
    LAST_RESULTS = res
    parts = []
    for o in res.results:
        oc = o["out"].reshape(128, NBT, DT)
        parts.append(np.ascontiguousarray(oc.transpose(1, 0, 2)).reshape(BP, DT))
    return np.concatenate(parts, axis=0).astype(np.float32)
